# revision 41
# baseline (speedup 1.0000x reference)
"""Trainium2 Bass kernel for nn_Attention (dual-softmax linear attention), fp8 version.

Reference computation (per batch b):
  q  = x @ Wq                    [S, DM]   (DM = H*DH = 1024)
  kv = x @ Wkv                   [S, 2*DM] -> per head h: cols [h*128, h*128+64) = k_h,
                                              cols [h*128+64, (h+1)*128) = v_h
  q  = softmax(q over dh) * DH^-0.5
  k  = softmax(k over s)
  ctx_h   = k_h^T @ v_h          [DH, DH]
  out_h   = q_h @ ctx_h          [S, DH]
  y  = out @ Wlin + blin         [S, DM]

Sharding: data-parallel over batch B=8 -> one batch element per NeuronCore.

v3: all three big GEMMs (kv-proj, q-proj, final projection) run in fp8e4m3
with MatmulPerfMode.DoubleRow (2 k-planes per instruction, ~1.5x PE rate).
Numerics held together by three tricks (validated in numpy, rel err ~8e-4):
  1. Host-side bias correction: the dominant fp8 error is the common-mode
     shift of v's column means from quantizing Wv. y_corr = SCALE *
     ((xbar @ (Wv - Wv8)) @ Wlin) is computed on host in fp64 and folded
     into blin. (k/q softmax invariances kill the Wk/Wq quant errors.)
  2. Centered W2: the folded weight W2 = blockdiag(ctx_n)@Wlin*SCALE is
     nearly constant along each head's 64 contraction rows, so its fp8
     quantization error is rank-1 and large. The kernel computes per-head
     column means Kbar on device, subtracts them before quantizing
     (W2c = W2 - Kbar), routes sum_h Kbar through an exact fp32 bias path
     (valid because softmax rows sum to 1), and phase B contracts eq8@W2c.
     This also kills the eq8 quantization noise hitting the constant part.
  3. Scales: x*2^7, W*2^10 (products 2^17, descaled in the exp/copy
     activations), qhat*2^7 (blkones=2^-7), W2c*2^16 -> phase-B psum 2^23,
     output y*2^17 in fp16, descaled on host.

Layouts: x is transposed and quantized on HOST (xT [D, S] fp8), removing
all device-side transposes. DoubleRow operands are [128, 2, F] pair tiles.
"""

import math

import numpy as np

import concourse.bass as bass
import concourse.mybir as mybir
from concourse import bacc
from concourse.tile import TileContext

F32 = mybir.dt.float32
F16 = mybir.dt.float16
F8 = mybir.dt.float8e4
AF = mybir.ActivationFunctionType
DR = mybir.MatmulPerfMode.DoubleRow

S, D = 4096, 1024
H, DH = 16, 64
DM = H * DH  # 1024
B = 8
SCALE = DH ** (-0.5)

P = 128          # partitions
NB = 512         # moving free-dim tile
NP = D // (2 * P)  # 4 k-pair tiles
NJ = DM // P     # 8 dout-tiles (head pairs)
HH = H // 2      # heads per kv half-tile

SX = 2.0 ** 7    # x fp8 scale
SW = 2.0 ** 10   # weight fp8 scale
DESC = 2.0 ** -17  # product descale
SW2 = 2.0 ** 16  # centered-W2 fp8 scale
SY = 2.0 ** 17   # output scale (fp16 out, descaled on host)


def build_nc(s_len=S):
    sc = s_len // NB
    nc = bacc.Bacc(None, target_bir_lowering=False)

    xt_in = nc.declare_dram_parameter("xT", [D, s_len], F8, isOutput=False)
    wq_in = nc.declare_dram_parameter("Wq8", [D, DM], F8, isOutput=False)
    wkv_in = nc.declare_dram_parameter("Wkv8", [D, 2 * DM], F8, isOutput=False)
    wlin_in = nc.declare_dram_parameter("Wlin", [DM, DM], F16, isOutput=False)
    blin_in = nc.declare_dram_parameter("blin17", [1, DM], F32, isOutput=False)
    y_out = nc.declare_dram_parameter("y", [s_len, DM], F16, isOutput=True)

    with TileContext(nc) as tc:
        from contextlib import ExitStack

        with ExitStack() as stk:
            consts = stk.enter_context(tc.tile_pool(name="consts", bufs=1))
            wpool = stk.enter_context(tc.tile_pool(name="wpool", bufs=1))

            # plane-masked fp8 blockdiag stationaries for the DR rowsum:
            # blk8[jo] contracts only k-plane jo of an e8 pair tile.
            blk8 = []
            for jo in range(2):
                t = consts.tile([P, 2, P], F8, tag=f"blk8_{jo}")
                nc.vector.memset(t, 0.0)
                nc.vector.memset(t[0:64, jo, 0:64], 2.0 ** -7)
                nc.vector.memset(t[64:128, jo, 64:128], 2.0 ** -7)
                blk8.append(t)
            ekbias = consts.tile([P, 1], F32, tag="ekbias")
            nc.vector.memset(ekbias, float(3 * math.log(2)))
            qebias = consts.tile([P, 1], F32, tag="qebias")
            nc.vector.memset(qebias, float(2 * math.log(2)))
            # per-head-half mean-broadcast [P,P] blockdiag(1/64) and all-1/64
            ones64 = consts.tile([P, P], F16, tag="ones64")
            nc.vector.memset(ones64, 1.0 / 64.0)
            # imb = I - blk64: one matmul bts^T @ imb yields the centered
            # AND transposed fold operand (Ac^T) directly
            imb = consts.tile([P, P], F16, tag="imb")
            nc.vector.memset(imb, 0.0)
            nc.vector.memset(imb[0:64, 0:64], -1.0 / 64.0)
            nc.vector.memset(imb[64:128, 64:128], -1.0 / 64.0)
            # set the diagonal to 1 - 1/64
            nc.gpsimd.affine_select(
                out=imb,
                in_=imb,
                compare_op=mybir.AluOpType.not_equal,
                fill=63.0 / 64.0,
                base=0,
                pattern=[[-1, P]],
                channel_multiplier=1,
            )
            # fold staging: zeroed once; the diag blocks are overwritten
            # per j, the off-diag stays 0 (blockdiag structure required)
            bts_tiles = []
            for k in range(4):
                t = consts.tile([P, P], F16, tag=f"bts{k}")
                nc.vector.memset(t, 0.0)
                bts_tiles.append(t)

            # bias broadcast to all partitions via step-0 partition DMA
            # (emitted at c==2 — the SWDGE transfer contends with the HW
            # queue for DMA engines, so keep it away from the startup)
            bias_bc = consts.tile([P, DM], F32, tag="bias_bc")

            def load_bias():
                blin_row = blin_in[0, :]
                blin_bcast_ap = bass.AP(
                    tensor=blin_row.tensor,
                    offset=blin_row.offset,
                    ap=[[0, P]] + list(blin_row.ap),
                )
                nc.gpsimd.dma_start(out=bias_bc, in_=blin_bcast_ap)

            # ctx accumulators (SBUF, fp32), TRANSPOSED layout:
            # ctx_acc[j][d, e] = sum_s ek[s,d] v[s,e], with the colsum of ek
            # landing in column 128 (fused into the ctx matmul via a ones
            # column in the v tile)
            ctx_acc = []
            cs_acc = []
            for j in range(NJ):
                ca = consts.tile([P, P + 1], F32, tag=f"ctx_acc{j}")
                nc.vector.memset(ca, 0.0)
                ctx_acc.append(ca)
                cs_acc.append(ca[:, P:P + 1])

            # resident weights: fp8 pair layout [128, 2, cols]
            wkv_sb = [
                wpool.tile([P, 2, 2 * DM], F8, tag=f"wkv{p}", name=f"wkv{p}")
                for p in range(NP)
            ]
            wq_sb = [
                wpool.tile([P, 2, DM], F8, tag=f"wq{p}", name=f"wq{p}")
                for p in range(NP)
            ]
            wlin_sb = [
                wpool.tile([P, DM], F16, tag=f"wlin{j}", name=f"wlin{j}")
                for j in range(NJ)
            ]

            def _pair_src(t_in, p, cols, c0, ncols):
                # [128, 2, ncols] DRAM view matching a pair tile: element
                # (part, i, col) -> row (2p+i)*128+part, col c0+col
                base = t_in[0:P, 0:1]
                return bass.AP(
                    tensor=base.tensor,
                    offset=(2 * p) * P * cols + c0,
                    ap=[[cols, P], [P * cols, 2], [1, ncols]],
                )

            # All loads go on the single sync HW queue in need-order: the
            # two HW DGE queues share the 16 DMA engines round-robin, so a
            # "background" queue steals bandwidth from the critical one.
            def load_wq_half(h):
                for p in range(NP):
                    nc.sync.dma_start(
                        out=wq_sb[p][:, :, h * NB:(h + 1) * NB],
                        in_=_pair_src(wq_in, p, DM, h * NB, NB),
                    )

            def load_wkv():
                for p in range(NP):
                    for h2 in range(2):
                        nc.sync.dma_start(
                            out=wkv_sb[p][:, :, h2 * DM:(h2 + 1) * DM],
                            in_=_pair_src(wkv_in, p, 2 * DM, h2 * DM, DM),
                        )

            def load_wlin():
                for j in range(NJ):
                    nc.sync.dma_start(
                        out=wlin_sb[j], in_=wlin_in[j * P:(j + 1) * P, :]
                    )

            xt_pool = stk.enter_context(tc.tile_pool(name="xt", bufs=3))
            ek_pool = stk.enter_context(tc.tile_pool(name="ek", bufs=1))
            vt_pool = stk.enter_context(tc.tile_pool(name="vt", bufs=1))
            e8p_pool = stk.enter_context(tc.tile_pool(name="e8p", bufs=1))
            rr_pool = stk.enter_context(tc.tile_pool(name="rr", bufs=2))
            eqres_pool = stk.enter_context(tc.tile_pool(name="eqres", bufs=1))
            eq8_res = [[None] * NP for _ in range(sc)]

            # wq first half on the sync queue, xt c0 right behind; the PE
            # starts once 1MB is in and then must never gap (HW power
            # management throttles the PE to 50% on activity onsets, so a
            # stop-start beginning retriggers the throttle window).
            load_wq_half(0)

            w2c8_sb = [None] * NP
            w2c8_pool = stk.enter_context(tc.tile_pool(name="w2c8", bufs=1))
            fsb_pool = stk.enter_context(tc.tile_pool(name="fsb", bufs=2))

            # ---------------- phase A ----------------
            with tc.tile_pool(name="qp", bufs=2, space="PSUM") as qp_pool:

                def q_block(c, xt, tail=False):
                    # q projection (DoubleRow fp8) -> e8 = 4*exp(q) (fp8).
                    # Per-pair DR rowsums (plane-masked blk8 stationaries)
                    # keep the whole q stream in one fp8-DR pipeline;
                    # eq8 = e8 * rr = softmax(q) * 2^7 exactly as before.
                    e8_tiles = [None] * NP

                    def flush_pair(jp):
                        for jo in range(2):
                            rsps = qp_pool.tile([P, NB], F32, tag="qp", name="rsps")
                            nc.tensor.matmul(
                                rsps, blk8[jo], e8_tiles[jp], perf_mode=DR
                            )
                            rr = rr_pool.tile([P, NB], F32, tag="rr", name="rr")
                            nc.vector.reciprocal_approx_fast(out=rr, in_=rsps)
                            # in the fold tail the DVE is the busiest engine;
                            # the normalize mul runs fine on idle GpSimd
                            eng = nc.gpsimd if tail else nc.vector
                            eng.tensor_mul(
                                eq8_res[c][jp][:, jo, :], e8_tiles[jp][:, jo, :], rr
                            )

                    for jp4 in range(NP):
                        for jo in range(2):
                            j = 2 * jp4 + jo
                            qps = qp_pool.tile([P, NB], F32, tag="qp", name="qps")
                            for p in range(NP):
                                nc.tensor.matmul(
                                    qps,
                                    wq_sb[p][:, :, j * P:(j + 1) * P],
                                    xt[p],
                                    start=(p == 0),
                                    stop=(p == NP - 1),
                                    perf_mode=DR,
                                )
                            if jo == 0:
                                e8_tiles[jp4] = e8p_pool.tile(
                                    [P, 2, NB], F8, tag=f"e8_{jp4}", name=f"e8_{jp4}"
                                )
                                eq8_res[c][jp4] = eqres_pool.tile(
                                    [P, 2, NB], F8, tag=f"eq{c}_{jp4}",
                                    name=f"eq{c}_{jp4}",
                                )
                            nc.scalar.activation(
                                e8_tiles[jp4][:, jo, :], qps, AF.Exp,
                                scale=DESC, bias=qebias,
                            )
                        if jp4 > 0:
                            flush_pair(jp4 - 1)
                    flush_pair(NP - 1)

                def kv_ctx_block(c, xt):
                    # kv projection (DoubleRow fp8), two 1024-wide halves.
                    # ek/v evacuate to fp8 pair tiles over t-parity so ctx
                    # also runs DoubleRow. ek = exp(kv*2^-17)*2^3 (bias=3ln2),
                    # v = kv*2^-17*2^5; psums descale 2^-8 at the ctx add.
                    ek_tiles = [[None, None] for _ in range(2)]
                    v_tiles = [[None, None] for _ in range(2)]
                    for t in range(4):
                        u, i = t // 2, t % 2
                        for h2 in range(2):
                            kvps = kvp_pool.tile([P, DM], F32, tag="kvp")
                            for p in range(NP):
                                for n in range(2):
                                    nc.tensor.matmul(
                                        kvps[:, n * NB:(n + 1) * NB],
                                        xt[p][:, :, t * P:(t + 1) * P],
                                        wkv_sb[p][
                                            :, :,
                                            h2 * DM + n * NB: h2 * DM + (n + 1) * NB,
                                        ],
                                        start=(p == 0),
                                        stop=(p == NP - 1),
                                        perf_mode=DR,
                                    )
                            kv3 = kvps.rearrange("p (h c) -> p h c", h=HH)
                            kv4 = kvps.rearrange(
                                "p (j g c) -> p j g c", j=HH // 2, g=2
                            )
                            if i == 0:
                                ek_tiles[u][h2] = ek_pool.tile(
                                    [P, 2, HH, DH], F8, tag=f"ek{u}_{h2}",
                                    name=f"ek{u}_{h2}",
                                )
                                # v pair tile with a fused ones column per
                                # head-pair: [P, 2, 4, 129]; col 128 = 32.0
                                # makes the ctx matmul also emit the ek
                                # colsum on its own output column.
                                v_tiles[u][h2] = vt_pool.tile(
                                    [P, 2, HH // 2, 2 * DH + 1], F8,
                                    tag=f"v{u}_{h2}", name=f"v{u}_{h2}",
                                )
                                if c == 0:
                                    nc.vector.memset(
                                        v_tiles[u][h2][:, :, :, 2 * DH:2 * DH + 1],
                                        32.0,
                                    )
                            nc.scalar.activation(
                                ek_tiles[u][h2][:, i],
                                kv3[:, :, 0:DH],
                                AF.Exp,
                                scale=DESC,
                                bias=ekbias,
                            )
                            nc.scalar.activation(
                                v_tiles[u][h2][:, i, :, 0:2 * DH].rearrange(
                                    "p j (g c) -> p j g c", g=2
                                ),
                                kv4[:, :, :, DH:2 * DH],
                                AF.Copy,
                                scale=DESC * 32.0,
                            )

                    # ctx accumulation (per head-pair j), fp8 DR, TRANSPOSED:
                    # stationary = ek slice -> out rows are k-features; the v
                    # tile's ones column makes out[:, 128] the ek colsum.
                    for j in range(NJ):
                        h2, jl = j // 4, j % 4
                        cps = ctxp_pool.tile([P, P + 4], F32, tag="ctxp")
                        for u in range(2):
                            ekf = ek_tiles[u][h2].rearrange("p two h c -> p two (h c)")
                            nc.tensor.matmul(
                                cps[:, 0:P + 1],
                                ekf[:, :, jl * P:(jl + 1) * P],
                                v_tiles[u][h2][:, :, jl, :],
                                start=(u == 0),
                                stop=(u == 1),
                                perf_mode=DR,
                            )
                        nc.vector.scalar_tensor_tensor(
                            out=ctx_acc[j][:, 0:P + 1],
                            in0=cps[:, 0:P + 1],
                            scalar=2.0 ** -8,
                            in1=ctx_acc[j][:, 0:P + 1],
                            op0=mybir.AluOpType.mult,
                            op1=mybir.AluOpType.add,
                        )

                with (
                    tc.tile_pool(name="kvp", bufs=2, space="PSUM") as kvp_pool,
                    tc.tile_pool(name="ctxp", bufs=2, space="PSUM") as ctxp_pool,
                ):
                    xt_tail = {}
                    for c in range(sc):
                        xt = []
                        for p in range(NP):
                            t8 = xt_pool.tile([P, 2, NB], F8, tag=f"xt{p}")
                            nc.sync.dma_start(
                                out=t8,
                                in_=_pair_src(xt_in, p, s_len, c * NB, NB),
                            )
                            xt.append(t8)
                        if c == 0:
                            load_wq_half(1)
                            load_wkv()
                        if c == 3:
                            load_wlin()
                        if c == 2:
                            load_bias()
                        if c < sc - 1:
                            q_block(c, xt)
                            kv_ctx_block(c, xt)
                        else:
                            # last chunk: kv/ctx only; its q block runs
                            # after ctx completes so the fold (which needs
                            # the full ctx) can hide in its matmul stream
                            kv_ctx_block(c, xt)
                            xt_tail[c] = xt

                # ------- fold: W2c = centered(rcs*ctx^T) @ Wlin, interleaved
                # with the last chunk's q block. Centering happens BEFORE the
                # Wlin contraction: bdc = Ac^T = bts^T @ (I - blk64) and
                # u_bc = bts^T @ ones64 come out of single fp16 matmuls in
                # exactly the orientation the W2c / bias contractions need.
                with (
                    tc.tile_pool(name="w2p", bufs=1, space="PSUM") as w2p_pool,
                    tc.tile_pool(name="ybp", bufs=1, space="PSUM") as ybp_pool,
                    tc.tile_pool(name="fpsA", bufs=1, space="PSUM") as fpsA_pool,
                ):
                    ybbc = ybp_pool.tile([P, DM], F32, tag="ybp")
                    # small fold psums packed as slices of one shared bank
                    bank32 = fpsA_pool.tile([P, 4 * P], F32, tag="bank32")
                    fs_ps = {}
                    fs_tiles = {}

                    def fold_s1(j):
                        csr = consts.tile([P, 1], F32, tag=f"csr{j}")
                        nc.vector.tensor_scalar(
                            out=csr,
                            in0=cs_acc[j],
                            scalar1=1.0 / (SCALE * SW2),
                            scalar2=None,
                            op0=mybir.AluOpType.mult,
                        )
                        rcs = consts.tile([P, 1], F32, tag=f"rcs{j}")
                        nc.vector.reciprocal_approx_fast(out=rcs, in_=csr)
                        bts = bts_tiles[j % 4]
                        nc.scalar.activation(
                            bts[0:64, 0:64], ctx_acc[j][0:64, 0:64],
                            AF.Copy, scale=rcs[0:64],
                        )
                        nc.scalar.activation(
                            bts[64:128, 64:128], ctx_acc[j][64:128, 64:128],
                            AF.Copy, scale=rcs[64:128],
                        )

                    def fold_s2(j):
                        # bdc = Ac^T (centered+transposed) and the broadcast
                        # bias row u, each via ONE fp16 matmul from bts
                        bts = bts_tiles[j % 4]
                        bdcps = bank32[:, (j % 2) * P:(j % 2 + 1) * P]
                        nc.tensor.matmul(bdcps, bts, imb)
                        ubps = bank32[:, (2 + j % 2) * P:(3 + j % 2) * P]
                        nc.tensor.matmul(ubps, bts, ones64)
                        fs_ps[j] = (bdcps, ubps)

                    def fold_s3(j):
                        bdcps, ubps = fs_ps.pop(j)
                        bdc = fsb_pool.tile([P, P], F16, tag="bdc")
                        nc.vector.tensor_copy(bdc, bdcps)
                        ut16 = fsb_pool.tile([P, P], F16, tag="ut16")
                        nc.vector.tensor_copy(ut16, ubps)
                        fs_tiles[j] = (bdc, ut16)

                    def fold_s4(j):
                        jp, jo = j // 2, j % 2
                        bdc, ut16 = fs_tiles.pop(j)
                        w2ps = w2p_pool.tile([P, DM], F32, tag="w2p")
                        for n in range(2):
                            nc.tensor.matmul(
                                w2ps[:, n * NB:(n + 1) * NB],
                                bdc,
                                wlin_sb[j][:, n * NB:(n + 1) * NB],
                            )
                        if jo == 0:
                            w2c8_sb[jp] = w2c8_pool.tile(
                                [P, 2, DM], F8, tag=f"w2c{jp}", name=f"w2c{jp}"
                            )
                        # split the evacs across ScalarE/DVE — the tail is
                        # engine-balance limited
                        if jo == 0:
                            nc.scalar.activation(
                                w2c8_sb[jp][:, jo, :], w2ps, AF.Copy, scale=1.0
                            )
                        else:
                            nc.vector.tensor_copy(w2c8_sb[jp][:, jo, :], w2ps)
                        for n in range(2):
                            nc.tensor.matmul(
                                ybbc[:, n * NB:(n + 1) * NB],
                                ut16,
                                wlin_sb[j][:, n * NB:(n + 1) * NB],
                                start=(j == 0),
                                stop=(j == NJ - 1),
                            )

                    # The tile scheduler is a readiness-driven priority list
                    # scheduler (emission order is NOT preserved). Emit the
                    # whole fold at priority 0 right after ctx completes:
                    # each fold op is picked the moment its deps resolve,
                    # and the q6/q7 matmul stream (emitted next) fills every
                    # gap in the fold's cross-engine chains.
                    # emission must still follow the true dataflow (slot
                    # reuse is tracked by emission order), so pipeline the
                    # stages: s1(t), s2(t-1), s3(t-2), s4(t-3)
                    with tc.high_priority():
                        for t in range(NJ + 3):
                            if t < NJ:
                                fold_s1(t)
                            if 0 <= t - 1 < NJ:
                                fold_s2(t - 1)
                            if 0 <= t - 2 < NJ:
                                fold_s3(t - 2)
                            if 0 <= t - 3 < NJ:
                                fold_s4(t - 3)

                    q_block(sc - 1, xt_tail[sc - 1], tail=True)
                    # bias = blin17_bc + 2 * ybbc  (2^17-scaled fp32)
                    nc.vector.scalar_tensor_tensor(
                        out=bias_bc,
                        in0=ybbc,
                        scalar=2.0,
                        in1=bias_bc,
                        op0=mybir.AluOpType.mult,
                        op1=mybir.AluOpType.add,
                    )

            y_pool = stk.enter_context(tc.tile_pool(name="ysb", bufs=3))

            # ---------------- phase B: final projection (DoubleRow fp8) ------
            with tc.tile_pool(name="yp", bufs=3, space="PSUM") as yp_pool:
                for c in range(sc):
                    for t in range(4):
                        yps = yp_pool.tile([P, DM], F32, tag="yp")
                        for jp in range(NP):
                            for n in range(2):
                                nc.tensor.matmul(
                                    yps[:, n * NB:(n + 1) * NB],
                                    eq8_res[c][jp][:, :, t * P:(t + 1) * P],
                                    w2c8_sb[jp][:, :, n * NB:(n + 1) * NB],
                                    start=(jp == 0),
                                    stop=(jp == NP - 1),
                                    perf_mode=DR,
                                )
                        ysb = y_pool.tile([P, DM], F16, tag="ysb")
                        nc.vector.scalar_tensor_tensor(
                            out=ysb,
                            in0=yps,
                            scalar=2.0 ** -6,
                            in1=bias_bc,
                            op0=mybir.AluOpType.mult,
                            op1=mybir.AluOpType.add,
                        )
                        # split each y store across both HW queues — a single
                        # dma_start runs on one DMA engine (~20GB/s) and the
                        # output stream otherwise falls behind phase B
                        r0 = c * NB + t * P
                        nc.sync.dma_start(
                            out=y_out[r0:r0 + 64, :], in_=ysb[0:64, :]
                        )
                        nc.scalar.dma_start(
                            out=y_out[r0 + 64:r0 + P, :], in_=ysb[64:128, :]
                        )
    nc.compile()
    return nc


def _q8(a, scale):
    import ml_dtypes
    return np.clip(
        np.asarray(a, dtype=np.float32) * scale, -240.0, 240.0
    ).astype(ml_dtypes.float8_e4m3)


def prepare_inputs(x, Wq, Wkv, Wlin, blin):
    """Host-side quantization, transpose, and bias correction. Returns in_maps."""
    x = np.asarray(x, dtype=np.float32)
    b = x.shape[0]
    wq8 = _q8(Wq, SW)
    wkv8 = _q8(Wkv, SW)
    wlin16 = np.asarray(Wlin, dtype=np.float32).astype(np.float16)
    blin32 = np.asarray(blin, dtype=np.float64).reshape(DM)

    # host bias correction: dominant fp8 error is the common-mode shift of
    # v column means from quantizing Wv; exact to first order in fp64.
    vcols = np.concatenate(
        [np.arange(h * 2 * DH + DH, (h + 1) * 2 * DH) for h in range(H)]
    )
    Wkv64 = np.asarray(Wkv, dtype=np.float64)
    Wkv8_deq = wkv8.astype(np.float32).astype(np.float64) / SW
    dWv = Wkv64[:, vcols] - Wkv8_deq[:, vcols]          # [D, H*DH]
    xbar = x.astype(np.float64).mean(axis=1)            # [b, D]
    y_corr = SCALE * ((xbar @ dWv) @ np.asarray(Wlin, dtype=np.float64))

    in_maps = []
    for i in range(b):
        x8t = _q8(np.ascontiguousarray(x[i].T), SX)     # [D, S] fp8
        blin17 = ((blin32 + y_corr[i]) * SY).astype(np.float32).reshape(1, DM)
        in_maps.append(
            {
                "xT": x8t,
                "Wq8": wq8,
                "Wkv8": wkv8,
                "Wlin": wlin16,
                "blin17": blin17,
            }
        )
    return in_maps


def kernel(x, Wq, Wkv, Wlin, blin):
    from concourse.bass_utils import run_bass_kernel_spmd

    x = np.asarray(x, dtype=np.float32)
    b = x.shape[0]
    nc = build_nc(x.shape[1])
    in_maps = prepare_inputs(x, Wq, Wkv, Wlin, blin)
    res = run_bass_kernel_spmd(nc, in_maps, list(range(b)))
    return np.stack(
        [res.results[i]["y"].astype(np.float32) for i in range(b)]
    ) * np.float32(1.0 / SY)


if __name__ == "__main__":
    rng = np.random.default_rng(0)
    x = rng.random((B, S, D), dtype=np.float32)
    Wq = (rng.standard_normal((D, DM)) * 0.02).astype(np.float32)
    Wkv = (rng.standard_normal((D, 2 * DM)) * 0.02).astype(np.float32)
    Wlin = (rng.standard_normal((DM, DM)) * 0.02).astype(np.float32)
    blin = np.zeros((DM,), dtype=np.float32)
    y = kernel(x=x, Wq=Wq, Wkv=Wkv, Wlin=Wlin, blin=blin)
    print(y.shape, y.dtype)



# revision 42
# speedup vs baseline: 1.0022x; 1.0022x over previous
"""Trainium2 Bass kernel for nn_Attention (dual-softmax linear attention), fp8 version.

Reference computation (per batch b):
  q  = x @ Wq                    [S, DM]   (DM = H*DH = 1024)
  kv = x @ Wkv                   [S, 2*DM] -> per head h: cols [h*128, h*128+64) = k_h,
                                              cols [h*128+64, (h+1)*128) = v_h
  q  = softmax(q over dh) * DH^-0.5
  k  = softmax(k over s)
  ctx_h   = k_h^T @ v_h          [DH, DH]
  out_h   = q_h @ ctx_h          [S, DH]
  y  = out @ Wlin + blin         [S, DM]

Sharding: data-parallel over batch B=8 -> one batch element per NeuronCore.

v3: all three big GEMMs (kv-proj, q-proj, final projection) run in fp8e4m3
with MatmulPerfMode.DoubleRow (2 k-planes per instruction, ~1.5x PE rate).
Numerics held together by three tricks (validated in numpy, rel err ~8e-4):
  1. Host-side bias correction: the dominant fp8 error is the common-mode
     shift of v's column means from quantizing Wv. y_corr = SCALE *
     ((xbar @ (Wv - Wv8)) @ Wlin) is computed on host in fp64 and folded
     into blin. (k/q softmax invariances kill the Wk/Wq quant errors.)
  2. Centered W2: the folded weight W2 = blockdiag(ctx_n)@Wlin*SCALE is
     nearly constant along each head's 64 contraction rows, so its fp8
     quantization error is rank-1 and large. The kernel computes per-head
     column means Kbar on device, subtracts them before quantizing
     (W2c = W2 - Kbar), routes sum_h Kbar through an exact fp32 bias path
     (valid because softmax rows sum to 1), and phase B contracts eq8@W2c.
     This also kills the eq8 quantization noise hitting the constant part.
  3. Scales: x*2^7, W*2^10 (products 2^17, descaled in the exp/copy
     activations), qhat*2^7 (blkones=2^-7), W2c*2^16 -> phase-B psum 2^23,
     output y*2^17 in fp16, descaled on host.

Layouts: x is transposed and quantized on HOST (xT [D, S] fp8), removing
all device-side transposes. DoubleRow operands are [128, 2, F] pair tiles.
"""

import math

import numpy as np

import concourse.bass as bass
import concourse.mybir as mybir
from concourse import bacc
from concourse.tile import TileContext

F32 = mybir.dt.float32
F16 = mybir.dt.float16
F8 = mybir.dt.float8e4
AF = mybir.ActivationFunctionType
DR = mybir.MatmulPerfMode.DoubleRow

S, D = 4096, 1024
H, DH = 16, 64
DM = H * DH  # 1024
B = 8
SCALE = DH ** (-0.5)

P = 128          # partitions
NB = 512         # moving free-dim tile
NP = D // (2 * P)  # 4 k-pair tiles
NJ = DM // P     # 8 dout-tiles (head pairs)
HH = H // 2      # heads per kv half-tile

SX = 2.0 ** 7    # x fp8 scale
SW = 2.0 ** 10   # weight fp8 scale
DESC = 2.0 ** -17  # product descale
SW2 = 2.0 ** 16  # centered-W2 fp8 scale
SY = 2.0 ** 17   # output scale (fp16 out, descaled on host)


def build_nc(s_len=S):
    sc = s_len // NB
    nc = bacc.Bacc(None, target_bir_lowering=False)

    xt_in = nc.declare_dram_parameter("xT", [D, s_len], F8, isOutput=False)
    wq_in = nc.declare_dram_parameter("Wq8", [D, DM], F8, isOutput=False)
    wkv_in = nc.declare_dram_parameter("Wkv8", [D, 2 * DM], F8, isOutput=False)
    wlin_in = nc.declare_dram_parameter("Wlin", [DM, DM], F16, isOutput=False)
    blin_in = nc.declare_dram_parameter("blin17", [1, DM], F32, isOutput=False)
    y_out = nc.declare_dram_parameter("y", [s_len, DM], F16, isOutput=True)

    with TileContext(nc) as tc:
        from contextlib import ExitStack

        with ExitStack() as stk:
            consts = stk.enter_context(tc.tile_pool(name="consts", bufs=1))
            wpool = stk.enter_context(tc.tile_pool(name="wpool", bufs=1))

            # plane-masked fp8 blockdiag stationaries for the DR rowsum:
            # blk8[jo] contracts only k-plane jo of an e8 pair tile.
            blk8 = []
            for jo in range(2):
                t = consts.tile([P, 2, P], F8, tag=f"blk8_{jo}")
                nc.vector.memset(t, 0.0)
                nc.vector.memset(t[0:64, jo, 0:64], 2.0 ** -7)
                nc.vector.memset(t[64:128, jo, 64:128], 2.0 ** -7)
                blk8.append(t)
            ekbias = consts.tile([P, 1], F32, tag="ekbias")
            nc.vector.memset(ekbias, float(3 * math.log(2)))
            qebias = consts.tile([P, 1], F32, tag="qebias")
            nc.vector.memset(qebias, float(2 * math.log(2)))
            # per-head-half mean-broadcast [P,P] blockdiag(1/64) and all-1/64
            ones64 = consts.tile([P, P], F16, tag="ones64")
            nc.vector.memset(ones64, 1.0 / 64.0)
            # imb = I - blk64: one matmul bts^T @ imb yields the centered
            # AND transposed fold operand (Ac^T) directly
            imb = consts.tile([P, P], F16, tag="imb")
            nc.vector.memset(imb, 0.0)
            nc.vector.memset(imb[0:64, 0:64], -1.0 / 64.0)
            nc.vector.memset(imb[64:128, 64:128], -1.0 / 64.0)
            # set the diagonal to 1 - 1/64
            nc.gpsimd.affine_select(
                out=imb,
                in_=imb,
                compare_op=mybir.AluOpType.not_equal,
                fill=63.0 / 64.0,
                base=0,
                pattern=[[-1, P]],
                channel_multiplier=1,
            )
            # fold staging: zeroed once; the diag blocks are overwritten
            # per j, the off-diag stays 0 (blockdiag structure required)
            bts_tiles = []
            for k in range(4):
                t = consts.tile([P, P], F16, tag=f"bts{k}")
                nc.vector.memset(t, 0.0)
                bts_tiles.append(t)

            # bias broadcast to all partitions via step-0 partition DMA
            # (emitted at c==2 — the SWDGE transfer contends with the HW
            # queue for DMA engines, so keep it away from the startup)
            bias_bc = consts.tile([P, DM], F32, tag="bias_bc")

            def load_bias():
                blin_row = blin_in[0, :]
                blin_bcast_ap = bass.AP(
                    tensor=blin_row.tensor,
                    offset=blin_row.offset,
                    ap=[[0, P]] + list(blin_row.ap),
                )
                nc.gpsimd.dma_start(out=bias_bc, in_=blin_bcast_ap)

            # ctx accumulators (SBUF, fp32), TRANSPOSED layout:
            # ctx_acc[j][d, e] = sum_s ek[s,d] v[s,e], with the colsum of ek
            # landing in column 128 (fused into the ctx matmul via a ones
            # column in the v tile)
            ctx_acc = []
            cs_acc = []
            for j in range(NJ):
                ca = consts.tile([P, P + 1], F32, tag=f"ctx_acc{j}")
                nc.vector.memset(ca, 0.0)
                ctx_acc.append(ca)
                cs_acc.append(ca[:, P:P + 1])

            # resident weights: fp8 pair layout [128, 2, cols]
            wkv_sb = [
                wpool.tile([P, 2, 2 * DM], F8, tag=f"wkv{p}", name=f"wkv{p}")
                for p in range(NP)
            ]
            wq_sb = [
                wpool.tile([P, 2, DM], F8, tag=f"wq{p}", name=f"wq{p}")
                for p in range(NP)
            ]
            wlin_sb = [
                wpool.tile([P, DM], F16, tag=f"wlin{j}", name=f"wlin{j}")
                for j in range(NJ)
            ]

            def _pair_src(t_in, p, cols, c0, ncols):
                # [128, 2, ncols] DRAM view matching a pair tile: element
                # (part, i, col) -> row (2p+i)*128+part, col c0+col
                base = t_in[0:P, 0:1]
                return bass.AP(
                    tensor=base.tensor,
                    offset=(2 * p) * P * cols + c0,
                    ap=[[cols, P], [P * cols, 2], [1, ncols]],
                )

            # All loads go on the single sync HW queue in need-order: the
            # two HW DGE queues share the 16 DMA engines round-robin, so a
            # "background" queue steals bandwidth from the critical one.
            def load_wq_half(h):
                for p in range(NP):
                    nc.sync.dma_start(
                        out=wq_sb[p][:, :, h * NB:(h + 1) * NB],
                        in_=_pair_src(wq_in, p, DM, h * NB, NB),
                    )

            def load_wkv():
                for p in range(NP):
                    for h2 in range(2):
                        nc.sync.dma_start(
                            out=wkv_sb[p][:, :, h2 * DM:(h2 + 1) * DM],
                            in_=_pair_src(wkv_in, p, 2 * DM, h2 * DM, DM),
                        )

            def load_wlin():
                for j in range(NJ):
                    nc.sync.dma_start(
                        out=wlin_sb[j], in_=wlin_in[j * P:(j + 1) * P, :]
                    )

            xt_pool = stk.enter_context(tc.tile_pool(name="xt", bufs=3))
            ek_pool = stk.enter_context(tc.tile_pool(name="ek", bufs=1))
            vt_pool = stk.enter_context(tc.tile_pool(name="vt", bufs=1))
            e8p_pool = stk.enter_context(tc.tile_pool(name="e8p", bufs=1))
            rr_pool = stk.enter_context(tc.tile_pool(name="rr", bufs=2))
            eqres_pool = stk.enter_context(tc.tile_pool(name="eqres", bufs=1))
            eq8_res = [[None] * NP for _ in range(sc)]

            # wq first half on the sync queue, xt c0 right behind; the PE
            # starts once 1MB is in and then must never gap (HW power
            # management throttles the PE to 50% on activity onsets, so a
            # stop-start beginning retriggers the throttle window).
            load_wq_half(0)

            w2c8_sb = [None] * NP
            w2c8_pool = stk.enter_context(tc.tile_pool(name="w2c8", bufs=1))
            fsb_pool = stk.enter_context(tc.tile_pool(name="fsb", bufs=2))

            # ---------------- phase A ----------------
            with tc.tile_pool(name="qp", bufs=2, space="PSUM") as qp_pool:

                def q_block(c, xt, tail=False):
                    # q projection (DoubleRow fp8) -> e8 = 4*exp(q) (fp8).
                    # Per-pair DR rowsums (plane-masked blk8 stationaries)
                    # keep the whole q stream in one fp8-DR pipeline;
                    # eq8 = e8 * rr = softmax(q) * 2^7 exactly as before.
                    e8_tiles = [None] * NP

                    def flush_pair(jp):
                        for jo in range(2):
                            rsps = qp_pool.tile([P, NB], F32, tag="qp", name="rsps")
                            nc.tensor.matmul(
                                rsps, blk8[jo], e8_tiles[jp], perf_mode=DR
                            )
                            rr = rr_pool.tile([P, NB], F32, tag="rr", name="rr")
                            nc.vector.reciprocal_approx_fast(out=rr, in_=rsps)
                            nc.vector.tensor_mul(
                                eq8_res[c][jp][:, jo, :], e8_tiles[jp][:, jo, :], rr
                            )

                    for jp4 in range(NP):
                        for jo in range(2):
                            j = 2 * jp4 + jo
                            qps = qp_pool.tile([P, NB], F32, tag="qp", name="qps")
                            for p in range(NP):
                                nc.tensor.matmul(
                                    qps,
                                    wq_sb[p][:, :, j * P:(j + 1) * P],
                                    xt[p],
                                    start=(p == 0),
                                    stop=(p == NP - 1),
                                    perf_mode=DR,
                                )
                            if jo == 0:
                                e8_tiles[jp4] = e8p_pool.tile(
                                    [P, 2, NB], F8, tag=f"e8_{jp4}", name=f"e8_{jp4}"
                                )
                                eq8_res[c][jp4] = eqres_pool.tile(
                                    [P, 2, NB], F8, tag=f"eq{c}_{jp4}",
                                    name=f"eq{c}_{jp4}",
                                )
                            nc.scalar.activation(
                                e8_tiles[jp4][:, jo, :], qps, AF.Exp,
                                scale=DESC, bias=qebias,
                            )
                        if jp4 > 0:
                            flush_pair(jp4 - 1)
                    flush_pair(NP - 1)

                def kv_ctx_block(c, xt):
                    # kv projection (DoubleRow fp8), two 1024-wide halves.
                    # ek/v evacuate to fp8 pair tiles over t-parity so ctx
                    # also runs DoubleRow. ek = exp(kv*2^-17)*2^3 (bias=3ln2),
                    # v = kv*2^-17*2^5; psums descale 2^-8 at the ctx add.
                    ek_tiles = [[None, None] for _ in range(2)]
                    v_tiles = [[None, None] for _ in range(2)]
                    for t in range(4):
                        u, i = t // 2, t % 2
                        for h2 in range(2):
                            kvps = kvp_pool.tile([P, DM], F32, tag="kvp")
                            for p in range(NP):
                                for n in range(2):
                                    nc.tensor.matmul(
                                        kvps[:, n * NB:(n + 1) * NB],
                                        xt[p][:, :, t * P:(t + 1) * P],
                                        wkv_sb[p][
                                            :, :,
                                            h2 * DM + n * NB: h2 * DM + (n + 1) * NB,
                                        ],
                                        start=(p == 0),
                                        stop=(p == NP - 1),
                                        perf_mode=DR,
                                    )
                            kv3 = kvps.rearrange("p (h c) -> p h c", h=HH)
                            kv4 = kvps.rearrange(
                                "p (j g c) -> p j g c", j=HH // 2, g=2
                            )
                            if i == 0:
                                ek_tiles[u][h2] = ek_pool.tile(
                                    [P, 2, HH, DH], F8, tag=f"ek{u}_{h2}",
                                    name=f"ek{u}_{h2}",
                                )
                                # v pair tile with a fused ones column per
                                # head-pair: [P, 2, 4, 129]; col 128 = 32.0
                                # makes the ctx matmul also emit the ek
                                # colsum on its own output column.
                                v_tiles[u][h2] = vt_pool.tile(
                                    [P, 2, HH // 2, 2 * DH + 1], F8,
                                    tag=f"v{u}_{h2}", name=f"v{u}_{h2}",
                                )
                                if c == 0:
                                    nc.vector.memset(
                                        v_tiles[u][h2][:, :, :, 2 * DH:2 * DH + 1],
                                        32.0,
                                    )
                            nc.scalar.activation(
                                ek_tiles[u][h2][:, i],
                                kv3[:, :, 0:DH],
                                AF.Exp,
                                scale=DESC,
                                bias=ekbias,
                            )
                            nc.scalar.activation(
                                v_tiles[u][h2][:, i, :, 0:2 * DH].rearrange(
                                    "p j (g c) -> p j g c", g=2
                                ),
                                kv4[:, :, :, DH:2 * DH],
                                AF.Copy,
                                scale=DESC * 32.0,
                            )

                    # ctx accumulation (per head-pair j), fp8 DR, TRANSPOSED:
                    # stationary = ek slice -> out rows are k-features; the v
                    # tile's ones column makes out[:, 128] the ek colsum.
                    for j in range(NJ):
                        h2, jl = j // 4, j % 4
                        cps = ctxp_pool.tile([P, P + 4], F32, tag="ctxp")
                        for u in range(2):
                            ekf = ek_tiles[u][h2].rearrange("p two h c -> p two (h c)")
                            nc.tensor.matmul(
                                cps[:, 0:P + 1],
                                ekf[:, :, jl * P:(jl + 1) * P],
                                v_tiles[u][h2][:, :, jl, :],
                                start=(u == 0),
                                stop=(u == 1),
                                perf_mode=DR,
                            )
                        nc.vector.scalar_tensor_tensor(
                            out=ctx_acc[j][:, 0:P + 1],
                            in0=cps[:, 0:P + 1],
                            scalar=2.0 ** -8,
                            in1=ctx_acc[j][:, 0:P + 1],
                            op0=mybir.AluOpType.mult,
                            op1=mybir.AluOpType.add,
                        )

                with (
                    tc.tile_pool(name="kvp", bufs=2, space="PSUM") as kvp_pool,
                    tc.tile_pool(name="ctxp", bufs=2, space="PSUM") as ctxp_pool,
                ):
                    xt_tail = {}
                    for c in range(sc):
                        xt = []
                        for p in range(NP):
                            t8 = xt_pool.tile([P, 2, NB], F8, tag=f"xt{p}")
                            nc.sync.dma_start(
                                out=t8,
                                in_=_pair_src(xt_in, p, s_len, c * NB, NB),
                            )
                            xt.append(t8)
                        if c == 0:
                            load_wq_half(1)
                            load_wkv()
                        if c == 3:
                            load_wlin()
                        if c == 2:
                            load_bias()
                        if c < sc - 1:
                            q_block(c, xt)
                            kv_ctx_block(c, xt)
                        else:
                            # last chunk: kv/ctx only; its q block runs
                            # after ctx completes so the fold (which needs
                            # the full ctx) can hide in its matmul stream
                            kv_ctx_block(c, xt)
                            xt_tail[c] = xt

                # ------- fold: W2c = centered(rcs*ctx^T) @ Wlin, interleaved
                # with the last chunk's q block. Centering happens BEFORE the
                # Wlin contraction: bdc = Ac^T = bts^T @ (I - blk64) and
                # u_bc = bts^T @ ones64 come out of single fp16 matmuls in
                # exactly the orientation the W2c / bias contractions need.
                with (
                    tc.tile_pool(name="w2p", bufs=1, space="PSUM") as w2p_pool,
                    tc.tile_pool(name="ybp", bufs=1, space="PSUM") as ybp_pool,
                    tc.tile_pool(name="fpsA", bufs=1, space="PSUM") as fpsA_pool,
                ):
                    ybbc = ybp_pool.tile([P, DM], F32, tag="ybp")
                    # small fold psums packed as slices of one shared bank
                    bank32 = fpsA_pool.tile([P, 4 * P], F32, tag="bank32")
                    fs_ps = {}
                    fs_tiles = {}

                    def fold_s1(j):
                        csr = consts.tile([P, 1], F32, tag=f"csr{j}")
                        nc.vector.tensor_scalar(
                            out=csr,
                            in0=cs_acc[j],
                            scalar1=1.0 / (SCALE * SW2),
                            scalar2=None,
                            op0=mybir.AluOpType.mult,
                        )
                        rcs = consts.tile([P, 1], F32, tag=f"rcs{j}")
                        nc.vector.reciprocal_approx_fast(out=rcs, in_=csr)
                        bts = bts_tiles[j % 4]
                        nc.scalar.activation(
                            bts[0:64, 0:64], ctx_acc[j][0:64, 0:64],
                            AF.Copy, scale=rcs[0:64],
                        )
                        nc.scalar.activation(
                            bts[64:128, 64:128], ctx_acc[j][64:128, 64:128],
                            AF.Copy, scale=rcs[64:128],
                        )

                    def fold_s2(j):
                        # bdc = Ac^T (centered+transposed) and the broadcast
                        # bias row u, each via ONE fp16 matmul from bts
                        bts = bts_tiles[j % 4]
                        bdcps = bank32[:, (j % 2) * P:(j % 2 + 1) * P]
                        nc.tensor.matmul(bdcps, bts, imb)
                        ubps = bank32[:, (2 + j % 2) * P:(3 + j % 2) * P]
                        nc.tensor.matmul(ubps, bts, ones64)
                        fs_ps[j] = (bdcps, ubps)

                    def fold_s3(j):
                        bdcps, ubps = fs_ps.pop(j)
                        bdc = fsb_pool.tile([P, P], F16, tag="bdc")
                        nc.vector.tensor_copy(bdc, bdcps)
                        ut16 = fsb_pool.tile([P, P], F16, tag="ut16")
                        nc.vector.tensor_copy(ut16, ubps)
                        fs_tiles[j] = (bdc, ut16)

                    def fold_s4(j):
                        jp, jo = j // 2, j % 2
                        bdc, ut16 = fs_tiles.pop(j)
                        w2ps = w2p_pool.tile([P, DM], F32, tag="w2p")
                        for n in range(2):
                            nc.tensor.matmul(
                                w2ps[:, n * NB:(n + 1) * NB],
                                bdc,
                                wlin_sb[j][:, n * NB:(n + 1) * NB],
                            )
                        if jo == 0:
                            w2c8_sb[jp] = w2c8_pool.tile(
                                [P, 2, DM], F8, tag=f"w2c{jp}", name=f"w2c{jp}"
                            )
                        # split the evacs across ScalarE/DVE — the tail is
                        # engine-balance limited
                        if jo == 0:
                            nc.scalar.activation(
                                w2c8_sb[jp][:, jo, :], w2ps, AF.Copy, scale=1.0
                            )
                        else:
                            nc.vector.tensor_copy(w2c8_sb[jp][:, jo, :], w2ps)
                        for n in range(2):
                            nc.tensor.matmul(
                                ybbc[:, n * NB:(n + 1) * NB],
                                ut16,
                                wlin_sb[j][:, n * NB:(n + 1) * NB],
                                start=(j == 0),
                                stop=(j == NJ - 1),
                            )

                    # The tile scheduler is a readiness-driven priority list
                    # scheduler (emission order is NOT preserved). Emit the
                    # whole fold at priority 0 right after ctx completes:
                    # each fold op is picked the moment its deps resolve,
                    # and the q6/q7 matmul stream (emitted next) fills every
                    # gap in the fold's cross-engine chains.
                    # emission must still follow the true dataflow (slot
                    # reuse is tracked by emission order), so pipeline the
                    # stages: s1(t), s2(t-1), s3(t-2), s4(t-3)
                    with tc.high_priority():
                        for t in range(NJ + 3):
                            if t < NJ:
                                fold_s1(t)
                            if 0 <= t - 1 < NJ:
                                fold_s2(t - 1)
                            if 0 <= t - 2 < NJ:
                                fold_s3(t - 2)
                            if 0 <= t - 3 < NJ:
                                fold_s4(t - 3)

                    q_block(sc - 1, xt_tail[sc - 1], tail=True)
                    # bias = blin17_bc + 2 * ybbc  (2^17-scaled fp32)
                    nc.vector.scalar_tensor_tensor(
                        out=bias_bc,
                        in0=ybbc,
                        scalar=2.0,
                        in1=bias_bc,
                        op0=mybir.AluOpType.mult,
                        op1=mybir.AluOpType.add,
                    )

            y_pool = stk.enter_context(tc.tile_pool(name="ysb", bufs=3))

            # ---------------- phase B: final projection (DoubleRow fp8) ------
            with tc.tile_pool(name="yp", bufs=3, space="PSUM") as yp_pool:
                for c in range(sc):
                    for t in range(4):
                        yps = yp_pool.tile([P, DM], F32, tag="yp")
                        for jp in range(NP):
                            for n in range(2):
                                nc.tensor.matmul(
                                    yps[:, n * NB:(n + 1) * NB],
                                    eq8_res[c][jp][:, :, t * P:(t + 1) * P],
                                    w2c8_sb[jp][:, :, n * NB:(n + 1) * NB],
                                    start=(jp == 0),
                                    stop=(jp == NP - 1),
                                    perf_mode=DR,
                                )
                        ysb = y_pool.tile([P, DM], F16, tag="ysb")
                        nc.vector.scalar_tensor_tensor(
                            out=ysb,
                            in0=yps,
                            scalar=2.0 ** -6,
                            in1=bias_bc,
                            op0=mybir.AluOpType.mult,
                            op1=mybir.AluOpType.add,
                        )
                        # split each y store across both HW queues — a single
                        # dma_start runs on one DMA engine (~20GB/s) and the
                        # output stream otherwise falls behind phase B
                        r0 = c * NB + t * P
                        nc.sync.dma_start(
                            out=y_out[r0:r0 + 64, :], in_=ysb[0:64, :]
                        )
                        nc.scalar.dma_start(
                            out=y_out[r0 + 64:r0 + P, :], in_=ysb[64:128, :]
                        )
    nc.compile()
    return nc


def _q8(a, scale):
    import ml_dtypes
    return np.clip(
        np.asarray(a, dtype=np.float32) * scale, -240.0, 240.0
    ).astype(ml_dtypes.float8_e4m3)


def prepare_inputs(x, Wq, Wkv, Wlin, blin):
    """Host-side quantization, transpose, and bias correction. Returns in_maps."""
    x = np.asarray(x, dtype=np.float32)
    b = x.shape[0]
    wq8 = _q8(Wq, SW)
    wkv8 = _q8(Wkv, SW)
    wlin16 = np.asarray(Wlin, dtype=np.float32).astype(np.float16)
    blin32 = np.asarray(blin, dtype=np.float64).reshape(DM)

    # host bias correction: dominant fp8 error is the common-mode shift of
    # v column means from quantizing Wv; exact to first order in fp64.
    vcols = np.concatenate(
        [np.arange(h * 2 * DH + DH, (h + 1) * 2 * DH) for h in range(H)]
    )
    Wkv64 = np.asarray(Wkv, dtype=np.float64)
    Wkv8_deq = wkv8.astype(np.float32).astype(np.float64) / SW
    dWv = Wkv64[:, vcols] - Wkv8_deq[:, vcols]          # [D, H*DH]
    xbar = x.astype(np.float64).mean(axis=1)            # [b, D]
    y_corr = SCALE * ((xbar @ dWv) @ np.asarray(Wlin, dtype=np.float64))

    in_maps = []
    for i in range(b):
        x8t = _q8(np.ascontiguousarray(x[i].T), SX)     # [D, S] fp8
        blin17 = ((blin32 + y_corr[i]) * SY).astype(np.float32).reshape(1, DM)
        in_maps.append(
            {
                "xT": x8t,
                "Wq8": wq8,
                "Wkv8": wkv8,
                "Wlin": wlin16,
                "blin17": blin17,
            }
        )
    return in_maps


def kernel(x, Wq, Wkv, Wlin, blin):
    from concourse.bass_utils import run_bass_kernel_spmd

    x = np.asarray(x, dtype=np.float32)
    b = x.shape[0]
    nc = build_nc(x.shape[1])
    in_maps = prepare_inputs(x, Wq, Wkv, Wlin, blin)
    res = run_bass_kernel_spmd(nc, in_maps, list(range(b)))
    return np.stack(
        [res.results[i]["y"].astype(np.float32) for i in range(b)]
    ) * np.float32(1.0 / SY)


if __name__ == "__main__":
    rng = np.random.default_rng(0)
    x = rng.random((B, S, D), dtype=np.float32)
    Wq = (rng.standard_normal((D, DM)) * 0.02).astype(np.float32)
    Wkv = (rng.standard_normal((D, 2 * DM)) * 0.02).astype(np.float32)
    Wlin = (rng.standard_normal((DM, DM)) * 0.02).astype(np.float32)
    blin = np.zeros((DM,), dtype=np.float32)
    y = kernel(x=x, Wq=Wq, Wkv=Wkv, Wlin=Wlin, blin=blin)
    print(y.shape, y.dtype)



# revision 43
# speedup vs baseline: 1.0032x; 1.0010x over previous
"""Trainium2 Bass kernel for nn_Attention (dual-softmax linear attention), fp8 version.

Reference computation (per batch b):
  q  = x @ Wq                    [S, DM]   (DM = H*DH = 1024)
  kv = x @ Wkv                   [S, 2*DM] -> per head h: cols [h*128, h*128+64) = k_h,
                                              cols [h*128+64, (h+1)*128) = v_h
  q  = softmax(q over dh) * DH^-0.5
  k  = softmax(k over s)
  ctx_h   = k_h^T @ v_h          [DH, DH]
  out_h   = q_h @ ctx_h          [S, DH]
  y  = out @ Wlin + blin         [S, DM]

Sharding: data-parallel over batch B=8 -> one batch element per NeuronCore.

v3: all three big GEMMs (kv-proj, q-proj, final projection) run in fp8e4m3
with MatmulPerfMode.DoubleRow (2 k-planes per instruction, ~1.5x PE rate).
Numerics held together by three tricks (validated in numpy, rel err ~8e-4):
  1. Host-side bias correction: the dominant fp8 error is the common-mode
     shift of v's column means from quantizing Wv. y_corr = SCALE *
     ((xbar @ (Wv - Wv8)) @ Wlin) is computed on host in fp64 and folded
     into blin. (k/q softmax invariances kill the Wk/Wq quant errors.)
  2. Centered W2: the folded weight W2 = blockdiag(ctx_n)@Wlin*SCALE is
     nearly constant along each head's 64 contraction rows, so its fp8
     quantization error is rank-1 and large. The kernel computes per-head
     column means Kbar on device, subtracts them before quantizing
     (W2c = W2 - Kbar), routes sum_h Kbar through an exact fp32 bias path
     (valid because softmax rows sum to 1), and phase B contracts eq8@W2c.
     This also kills the eq8 quantization noise hitting the constant part.
  3. Scales: x*2^7, W*2^10 (products 2^17, descaled in the exp/copy
     activations), qhat*2^7 (blkones=2^-7), W2c*2^16 -> phase-B psum 2^23,
     output y*2^17 in fp16, descaled on host.

Layouts: x is transposed and quantized on HOST (xT [D, S] fp8), removing
all device-side transposes. DoubleRow operands are [128, 2, F] pair tiles.
"""

import math

import numpy as np

import concourse.bass as bass
import concourse.mybir as mybir
from concourse import bacc
from concourse.tile import TileContext

F32 = mybir.dt.float32
F16 = mybir.dt.float16
F8 = mybir.dt.float8e4
AF = mybir.ActivationFunctionType
DR = mybir.MatmulPerfMode.DoubleRow

S, D = 4096, 1024
H, DH = 16, 64
DM = H * DH  # 1024
B = 8
SCALE = DH ** (-0.5)

P = 128          # partitions
NB = 512         # moving free-dim tile
NP = D // (2 * P)  # 4 k-pair tiles
NJ = DM // P     # 8 dout-tiles (head pairs)
HH = H // 2      # heads per kv half-tile

SX = 2.0 ** 7    # x fp8 scale
SW = 2.0 ** 10   # weight fp8 scale
DESC = 2.0 ** -17  # product descale
SW2 = 2.0 ** 16  # centered-W2 fp8 scale
SY = 2.0 ** 17   # output scale (fp16 out, descaled on host)


def build_nc(s_len=S):
    sc = s_len // NB
    nc = bacc.Bacc(None, target_bir_lowering=False)

    xt_in = nc.declare_dram_parameter("xT", [D, s_len], F8, isOutput=False)
    wq_in = nc.declare_dram_parameter("Wq8", [D, DM], F8, isOutput=False)
    wkv_in = nc.declare_dram_parameter("Wkv8", [D, 2 * DM], F8, isOutput=False)
    wlin_in = nc.declare_dram_parameter("Wlin", [DM, DM], F16, isOutput=False)
    blin_in = nc.declare_dram_parameter("blin17", [1, DM], F32, isOutput=False)
    y_out = nc.declare_dram_parameter("y", [s_len, DM], F16, isOutput=True)

    with TileContext(nc) as tc:
        from contextlib import ExitStack

        with ExitStack() as stk:
            consts = stk.enter_context(tc.tile_pool(name="consts", bufs=1))
            wpool = stk.enter_context(tc.tile_pool(name="wpool", bufs=1))

            # plane-masked fp8 blockdiag stationaries for the DR rowsum:
            # blk8[jo] contracts only k-plane jo of an e8 pair tile.
            blk8 = []
            for jo in range(2):
                t = consts.tile([P, 2, P], F8, tag=f"blk8_{jo}")
                nc.vector.memset(t, 0.0)
                nc.vector.memset(t[0:64, jo, 0:64], 2.0 ** -7)
                nc.vector.memset(t[64:128, jo, 64:128], 2.0 ** -7)
                blk8.append(t)
            ekbias = consts.tile([P, 1], F32, tag="ekbias")
            nc.vector.memset(ekbias, float(3 * math.log(2)))
            qebias = consts.tile([P, 1], F32, tag="qebias")
            nc.vector.memset(qebias, float(2 * math.log(2)))
            # per-head-half mean-broadcast [P,P] blockdiag(1/64) and all-1/64
            ones64 = consts.tile([P, P], F16, tag="ones64")
            nc.vector.memset(ones64, 1.0 / 64.0)
            # imb = I - blk64: one matmul bts^T @ imb yields the centered
            # AND transposed fold operand (Ac^T) directly
            imb = consts.tile([P, P], F16, tag="imb")
            nc.vector.memset(imb, 0.0)
            nc.vector.memset(imb[0:64, 0:64], -1.0 / 64.0)
            nc.vector.memset(imb[64:128, 64:128], -1.0 / 64.0)
            # set the diagonal to 1 - 1/64
            nc.gpsimd.affine_select(
                out=imb,
                in_=imb,
                compare_op=mybir.AluOpType.not_equal,
                fill=63.0 / 64.0,
                base=0,
                pattern=[[-1, P]],
                channel_multiplier=1,
            )
            # fold staging: zeroed once; the diag blocks are overwritten
            # per j, the off-diag stays 0 (blockdiag structure required)
            bts_tiles = []
            for k in range(4):
                t = consts.tile([P, P], F16, tag=f"bts{k}")
                nc.vector.memset(t, 0.0)
                bts_tiles.append(t)

            # bias broadcast to all partitions via step-0 partition DMA
            # (emitted at c==2 — the SWDGE transfer contends with the HW
            # queue for DMA engines, so keep it away from the startup)
            bias_bc = consts.tile([P, DM], F32, tag="bias_bc")

            def load_bias():
                blin_row = blin_in[0, :]
                blin_bcast_ap = bass.AP(
                    tensor=blin_row.tensor,
                    offset=blin_row.offset,
                    ap=[[0, P]] + list(blin_row.ap),
                )
                nc.gpsimd.dma_start(out=bias_bc, in_=blin_bcast_ap)

            # ctx accumulators (SBUF, fp32), TRANSPOSED layout:
            # ctx_acc[j][d, e] = sum_s ek[s,d] v[s,e], with the colsum of ek
            # landing in column 128 (fused into the ctx matmul via a ones
            # column in the v tile)
            ctx_acc = []
            cs_acc = []
            for j in range(NJ):
                ca = consts.tile([P, P + 1], F32, tag=f"ctx_acc{j}")
                nc.vector.memset(ca, 0.0)
                ctx_acc.append(ca)
                cs_acc.append(ca[:, P:P + 1])

            # resident weights: fp8 pair layout [128, 2, cols]
            wkv_sb = [
                wpool.tile([P, 2, 2 * DM], F8, tag=f"wkv{p}", name=f"wkv{p}")
                for p in range(NP)
            ]
            wq_sb = [
                wpool.tile([P, 2, DM], F8, tag=f"wq{p}", name=f"wq{p}")
                for p in range(NP)
            ]
            wlin_sb = [
                wpool.tile([P, DM], F16, tag=f"wlin{j}", name=f"wlin{j}")
                for j in range(NJ)
            ]

            def _pair_src(t_in, p, cols, c0, ncols):
                # [128, 2, ncols] DRAM view matching a pair tile: element
                # (part, i, col) -> row (2p+i)*128+part, col c0+col
                base = t_in[0:P, 0:1]
                return bass.AP(
                    tensor=base.tensor,
                    offset=(2 * p) * P * cols + c0,
                    ap=[[cols, P], [P * cols, 2], [1, ncols]],
                )

            # All loads go on the single sync HW queue in need-order: the
            # two HW DGE queues share the 16 DMA engines round-robin, so a
            # "background" queue steals bandwidth from the critical one.
            def load_wq_half(h):
                for p in range(NP):
                    nc.sync.dma_start(
                        out=wq_sb[p][:, :, h * NB:(h + 1) * NB],
                        in_=_pair_src(wq_in, p, DM, h * NB, NB),
                    )

            def load_wkv():
                for p in range(NP):
                    for h2 in range(2):
                        nc.sync.dma_start(
                            out=wkv_sb[p][:, :, h2 * DM:(h2 + 1) * DM],
                            in_=_pair_src(wkv_in, p, 2 * DM, h2 * DM, DM),
                        )

            def load_wlin():
                for j in range(NJ):
                    nc.sync.dma_start(
                        out=wlin_sb[j], in_=wlin_in[j * P:(j + 1) * P, :]
                    )

            xt_pool = stk.enter_context(tc.tile_pool(name="xt", bufs=3))
            ek_pool = stk.enter_context(tc.tile_pool(name="ek", bufs=1))
            vt_pool = stk.enter_context(tc.tile_pool(name="vt", bufs=1))
            e8p_pool = stk.enter_context(tc.tile_pool(name="e8p", bufs=1))
            rr_pool = stk.enter_context(tc.tile_pool(name="rr", bufs=2))
            eqres_pool = stk.enter_context(tc.tile_pool(name="eqres", bufs=1))
            eq8_res = [[None] * NP for _ in range(sc)]

            # wq first half on the sync queue, xt c0 right behind; the PE
            # starts once 1MB is in and then must never gap (HW power
            # management throttles the PE to 50% on activity onsets, so a
            # stop-start beginning retriggers the throttle window).
            load_wq_half(0)

            w2c8_sb = [None] * NP
            w2c8_pool = stk.enter_context(tc.tile_pool(name="w2c8", bufs=1))
            fsb_pool = stk.enter_context(tc.tile_pool(name="fsb", bufs=2))

            # ---------------- phase A ----------------
            with tc.tile_pool(name="qp", bufs=2, space="PSUM") as qp_pool:

                def q_block(c, xt, tail=False):
                    # q projection (DoubleRow fp8) -> e8 = 4*exp(q) (fp8).
                    # Per-pair DR rowsums (plane-masked blk8 stationaries)
                    # keep the whole q stream in one fp8-DR pipeline;
                    # eq8 = e8 * rr = softmax(q) * 2^7 exactly as before.
                    e8_tiles = [None] * NP

                    def flush_pair(jp):
                        for jo in range(2):
                            rsps = qp_pool.tile([P, NB], F32, tag="qp", name="rsps")
                            nc.tensor.matmul(
                                rsps, blk8[jo], e8_tiles[jp], perf_mode=DR
                            )
                            rr = rr_pool.tile([P, NB], F32, tag="rr", name="rr")
                            nc.vector.reciprocal_approx_fast(out=rr, in_=rsps)
                            nc.vector.tensor_mul(
                                eq8_res[c][jp][:, jo, :], e8_tiles[jp][:, jo, :], rr
                            )

                    for jp4 in range(NP):
                        for jo in range(2):
                            j = 2 * jp4 + jo
                            qps = qp_pool.tile([P, NB], F32, tag="qp", name="qps")
                            for p in range(NP):
                                nc.tensor.matmul(
                                    qps,
                                    wq_sb[p][:, :, j * P:(j + 1) * P],
                                    xt[p],
                                    start=(p == 0),
                                    stop=(p == NP - 1),
                                    perf_mode=DR,
                                )
                            if jo == 0:
                                e8_tiles[jp4] = e8p_pool.tile(
                                    [P, 2, NB], F8, tag=f"e8_{jp4}", name=f"e8_{jp4}"
                                )
                                eq8_res[c][jp4] = eqres_pool.tile(
                                    [P, 2, NB], F8, tag=f"eq{c}_{jp4}",
                                    name=f"eq{c}_{jp4}",
                                )
                            nc.scalar.activation(
                                e8_tiles[jp4][:, jo, :], qps, AF.Exp,
                                scale=DESC, bias=qebias,
                            )
                        if jp4 > 0:
                            flush_pair(jp4 - 1)
                    flush_pair(NP - 1)

                def kv_ctx_block(c, xt):
                    # kv projection (DoubleRow fp8), two 1024-wide halves.
                    # ek/v evacuate to fp8 pair tiles over t-parity so ctx
                    # also runs DoubleRow. ek = exp(kv*2^-17)*2^3 (bias=3ln2),
                    # v = kv*2^-17*2^5; psums descale 2^-8 at the ctx add.
                    ek_tiles = [[None, None] for _ in range(2)]
                    v_tiles = [[None, None] for _ in range(2)]
                    for t in range(4):
                        u, i = t // 2, t % 2
                        for h2 in range(2):
                            kvps = kvp_pool.tile([P, DM], F32, tag="kvp")
                            for p in range(NP):
                                for n in range(2):
                                    nc.tensor.matmul(
                                        kvps[:, n * NB:(n + 1) * NB],
                                        xt[p][:, :, t * P:(t + 1) * P],
                                        wkv_sb[p][
                                            :, :,
                                            h2 * DM + n * NB: h2 * DM + (n + 1) * NB,
                                        ],
                                        start=(p == 0),
                                        stop=(p == NP - 1),
                                        perf_mode=DR,
                                    )
                            kv3 = kvps.rearrange("p (h c) -> p h c", h=HH)
                            kv4 = kvps.rearrange(
                                "p (j g c) -> p j g c", j=HH // 2, g=2
                            )
                            if i == 0:
                                ek_tiles[u][h2] = ek_pool.tile(
                                    [P, 2, HH, DH], F8, tag=f"ek{u}_{h2}",
                                    name=f"ek{u}_{h2}",
                                )
                                # v pair tile with a fused ones column per
                                # head-pair: [P, 2, 4, 129]; col 128 = 32.0
                                # makes the ctx matmul also emit the ek
                                # colsum on its own output column.
                                v_tiles[u][h2] = vt_pool.tile(
                                    [P, 2, HH // 2, 2 * DH + 1], F8,
                                    tag=f"v{u}_{h2}", name=f"v{u}_{h2}",
                                )
                                if c == 0:
                                    nc.vector.memset(
                                        v_tiles[u][h2][:, :, :, 2 * DH:2 * DH + 1],
                                        32.0,
                                    )
                            nc.scalar.activation(
                                ek_tiles[u][h2][:, i],
                                kv3[:, :, 0:DH],
                                AF.Exp,
                                scale=DESC,
                                bias=ekbias,
                            )
                            nc.scalar.activation(
                                v_tiles[u][h2][:, i, :, 0:2 * DH].rearrange(
                                    "p j (g c) -> p j g c", g=2
                                ),
                                kv4[:, :, :, DH:2 * DH],
                                AF.Copy,
                                scale=DESC * 32.0,
                            )

                    # ctx accumulation (per head-pair j), fp8 DR, TRANSPOSED:
                    # stationary = ek slice -> out rows are k-features; the v
                    # tile's ones column makes out[:, 128] the ek colsum.
                    for j in range(NJ):
                        h2, jl = j // 4, j % 4
                        cps = ctxp_pool.tile([P, P + 4], F32, tag="ctxp")
                        for u in range(2):
                            ekf = ek_tiles[u][h2].rearrange("p two h c -> p two (h c)")
                            nc.tensor.matmul(
                                cps[:, 0:P + 1],
                                ekf[:, :, jl * P:(jl + 1) * P],
                                v_tiles[u][h2][:, :, jl, :],
                                start=(u == 0),
                                stop=(u == 1),
                                perf_mode=DR,
                            )
                        nc.vector.scalar_tensor_tensor(
                            out=ctx_acc[j][:, 0:P + 1],
                            in0=cps[:, 0:P + 1],
                            scalar=2.0 ** -8,
                            in1=ctx_acc[j][:, 0:P + 1],
                            op0=mybir.AluOpType.mult,
                            op1=mybir.AluOpType.add,
                        )

                with (
                    tc.tile_pool(name="kvp", bufs=2, space="PSUM") as kvp_pool,
                    tc.tile_pool(name="ctxp", bufs=2, space="PSUM") as ctxp_pool,
                ):
                    xt_tail = {}
                    for c in range(sc):
                        xt = []
                        for p in range(NP):
                            t8 = xt_pool.tile([P, 2, NB], F8, tag=f"xt{p}")
                            nc.sync.dma_start(
                                out=t8,
                                in_=_pair_src(xt_in, p, s_len, c * NB, NB),
                            )
                            xt.append(t8)
                        if c == 0:
                            load_wq_half(1)
                            load_wkv()
                        if c == 1:
                            load_wlin()
                        if c == 2:
                            load_bias()
                        if c < sc - 1:
                            q_block(c, xt)
                            kv_ctx_block(c, xt)
                        else:
                            # last chunk: kv/ctx only; its q block runs
                            # after ctx completes so the fold (which needs
                            # the full ctx) can hide in its matmul stream
                            kv_ctx_block(c, xt)
                            xt_tail[c] = xt

                # ------- fold: W2c = centered(rcs*ctx^T) @ Wlin, interleaved
                # with the last chunk's q block. Centering happens BEFORE the
                # Wlin contraction: bdc = Ac^T = bts^T @ (I - blk64) and
                # u_bc = bts^T @ ones64 come out of single fp16 matmuls in
                # exactly the orientation the W2c / bias contractions need.
                with (
                    tc.tile_pool(name="w2p", bufs=1, space="PSUM") as w2p_pool,
                    tc.tile_pool(name="ybp", bufs=1, space="PSUM") as ybp_pool,
                    tc.tile_pool(name="fpsA", bufs=1, space="PSUM") as fpsA_pool,
                ):
                    ybbc = ybp_pool.tile([P, DM], F32, tag="ybp")
                    # small fold psums packed as slices of one shared bank
                    bank32 = fpsA_pool.tile([P, 4 * P], F32, tag="bank32")
                    fs_ps = {}
                    fs_tiles = {}

                    def fold_s1(j):
                        csr = consts.tile([P, 1], F32, tag=f"csr{j}")
                        nc.vector.tensor_scalar(
                            out=csr,
                            in0=cs_acc[j],
                            scalar1=1.0 / (SCALE * SW2),
                            scalar2=None,
                            op0=mybir.AluOpType.mult,
                        )
                        rcs = consts.tile([P, 1], F32, tag=f"rcs{j}")
                        nc.vector.reciprocal_approx_fast(out=rcs, in_=csr)
                        bts = bts_tiles[j % 4]
                        nc.scalar.activation(
                            bts[0:64, 0:64], ctx_acc[j][0:64, 0:64],
                            AF.Copy, scale=rcs[0:64],
                        )
                        nc.scalar.activation(
                            bts[64:128, 64:128], ctx_acc[j][64:128, 64:128],
                            AF.Copy, scale=rcs[64:128],
                        )

                    def fold_s2(j):
                        # bdc = Ac^T (centered+transposed) and the broadcast
                        # bias row u, each via ONE fp16 matmul from bts
                        bts = bts_tiles[j % 4]
                        bdcps = bank32[:, (j % 2) * P:(j % 2 + 1) * P]
                        nc.tensor.matmul(bdcps, bts, imb)
                        ubps = bank32[:, (2 + j % 2) * P:(3 + j % 2) * P]
                        nc.tensor.matmul(ubps, bts, ones64)
                        fs_ps[j] = (bdcps, ubps)

                    def fold_s3(j):
                        bdcps, ubps = fs_ps.pop(j)
                        bdc = fsb_pool.tile([P, P], F16, tag="bdc")
                        nc.vector.tensor_copy(bdc, bdcps)
                        ut16 = fsb_pool.tile([P, P], F16, tag="ut16")
                        nc.vector.tensor_copy(ut16, ubps)
                        fs_tiles[j] = (bdc, ut16)

                    def fold_s4(j):
                        jp, jo = j // 2, j % 2
                        bdc, ut16 = fs_tiles.pop(j)
                        w2ps = w2p_pool.tile([P, DM], F32, tag="w2p")
                        for n in range(2):
                            nc.tensor.matmul(
                                w2ps[:, n * NB:(n + 1) * NB],
                                bdc,
                                wlin_sb[j][:, n * NB:(n + 1) * NB],
                            )
                        if jo == 0:
                            w2c8_sb[jp] = w2c8_pool.tile(
                                [P, 2, DM], F8, tag=f"w2c{jp}", name=f"w2c{jp}"
                            )
                        # split the evacs across ScalarE/DVE — the tail is
                        # engine-balance limited
                        if jo == 0:
                            nc.scalar.activation(
                                w2c8_sb[jp][:, jo, :], w2ps, AF.Copy, scale=1.0
                            )
                        else:
                            nc.vector.tensor_copy(w2c8_sb[jp][:, jo, :], w2ps)
                        for n in range(2):
                            nc.tensor.matmul(
                                ybbc[:, n * NB:(n + 1) * NB],
                                ut16,
                                wlin_sb[j][:, n * NB:(n + 1) * NB],
                                start=(j == 0),
                                stop=(j == NJ - 1),
                            )

                    # The tile scheduler is a readiness-driven priority list
                    # scheduler (emission order is NOT preserved). Emit the
                    # whole fold at priority 0 right after ctx completes:
                    # each fold op is picked the moment its deps resolve,
                    # and the q6/q7 matmul stream (emitted next) fills every
                    # gap in the fold's cross-engine chains.
                    # emission must still follow the true dataflow (slot
                    # reuse is tracked by emission order), so pipeline the
                    # stages: s1(t), s2(t-1), s3(t-2), s4(t-3)
                    with tc.high_priority():
                        for t in range(NJ + 3):
                            if t < NJ:
                                fold_s1(t)
                            if 0 <= t - 1 < NJ:
                                fold_s2(t - 1)
                            if 0 <= t - 2 < NJ:
                                fold_s3(t - 2)
                            if 0 <= t - 3 < NJ:
                                fold_s4(t - 3)

                    q_block(sc - 1, xt_tail[sc - 1], tail=True)
                    # bias = blin17_bc + 2 * ybbc  (2^17-scaled fp32)
                    nc.vector.scalar_tensor_tensor(
                        out=bias_bc,
                        in0=ybbc,
                        scalar=2.0,
                        in1=bias_bc,
                        op0=mybir.AluOpType.mult,
                        op1=mybir.AluOpType.add,
                    )

            y_pool = stk.enter_context(tc.tile_pool(name="ysb", bufs=3))

            # ---------------- phase B: final projection (DoubleRow fp8) ------
            with tc.tile_pool(name="yp", bufs=3, space="PSUM") as yp_pool:
                for c in range(sc):
                    for t in range(4):
                        yps = yp_pool.tile([P, DM], F32, tag="yp")
                        for jp in range(NP):
                            for n in range(2):
                                nc.tensor.matmul(
                                    yps[:, n * NB:(n + 1) * NB],
                                    eq8_res[c][jp][:, :, t * P:(t + 1) * P],
                                    w2c8_sb[jp][:, :, n * NB:(n + 1) * NB],
                                    start=(jp == 0),
                                    stop=(jp == NP - 1),
                                    perf_mode=DR,
                                )
                        ysb = y_pool.tile([P, DM], F16, tag="ysb")
                        nc.vector.scalar_tensor_tensor(
                            out=ysb,
                            in0=yps,
                            scalar=2.0 ** -6,
                            in1=bias_bc,
                            op0=mybir.AluOpType.mult,
                            op1=mybir.AluOpType.add,
                        )
                        # split each y store across both HW queues — a single
                        # dma_start runs on one DMA engine (~20GB/s) and the
                        # output stream otherwise falls behind phase B
                        r0 = c * NB + t * P
                        nc.sync.dma_start(
                            out=y_out[r0:r0 + 64, :], in_=ysb[0:64, :]
                        )
                        nc.scalar.dma_start(
                            out=y_out[r0 + 64:r0 + P, :], in_=ysb[64:128, :]
                        )
    nc.compile()
    return nc


def _q8(a, scale):
    import ml_dtypes
    return np.clip(
        np.asarray(a, dtype=np.float32) * scale, -240.0, 240.0
    ).astype(ml_dtypes.float8_e4m3)


def prepare_inputs(x, Wq, Wkv, Wlin, blin):
    """Host-side quantization, transpose, and bias correction. Returns in_maps."""
    x = np.asarray(x, dtype=np.float32)
    b = x.shape[0]
    wq8 = _q8(Wq, SW)
    wkv8 = _q8(Wkv, SW)
    wlin16 = np.asarray(Wlin, dtype=np.float32).astype(np.float16)
    blin32 = np.asarray(blin, dtype=np.float64).reshape(DM)

    # host bias correction: dominant fp8 error is the common-mode shift of
    # v column means from quantizing Wv; exact to first order in fp64.
    vcols = np.concatenate(
        [np.arange(h * 2 * DH + DH, (h + 1) * 2 * DH) for h in range(H)]
    )
    Wkv64 = np.asarray(Wkv, dtype=np.float64)
    Wkv8_deq = wkv8.astype(np.float32).astype(np.float64) / SW
    dWv = Wkv64[:, vcols] - Wkv8_deq[:, vcols]          # [D, H*DH]
    xbar = x.astype(np.float64).mean(axis=1)            # [b, D]
    y_corr = SCALE * ((xbar @ dWv) @ np.asarray(Wlin, dtype=np.float64))

    in_maps = []
    for i in range(b):
        x8t = _q8(np.ascontiguousarray(x[i].T), SX)     # [D, S] fp8
        blin17 = ((blin32 + y_corr[i]) * SY).astype(np.float32).reshape(1, DM)
        in_maps.append(
            {
                "xT": x8t,
                "Wq8": wq8,
                "Wkv8": wkv8,
                "Wlin": wlin16,
                "blin17": blin17,
            }
        )
    return in_maps


def kernel(x, Wq, Wkv, Wlin, blin):
    from concourse.bass_utils import run_bass_kernel_spmd

    x = np.asarray(x, dtype=np.float32)
    b = x.shape[0]
    nc = build_nc(x.shape[1])
    in_maps = prepare_inputs(x, Wq, Wkv, Wlin, blin)
    res = run_bass_kernel_spmd(nc, in_maps, list(range(b)))
    return np.stack(
        [res.results[i]["y"].astype(np.float32) for i in range(b)]
    ) * np.float32(1.0 / SY)


if __name__ == "__main__":
    rng = np.random.default_rng(0)
    x = rng.random((B, S, D), dtype=np.float32)
    Wq = (rng.standard_normal((D, DM)) * 0.02).astype(np.float32)
    Wkv = (rng.standard_normal((D, 2 * DM)) * 0.02).astype(np.float32)
    Wlin = (rng.standard_normal((DM, DM)) * 0.02).astype(np.float32)
    blin = np.zeros((DM,), dtype=np.float32)
    y = kernel(x=x, Wq=Wq, Wkv=Wkv, Wlin=Wlin, blin=blin)
    print(y.shape, y.dtype)

